# revision 2
# baseline (speedup 1.0000x reference)
"""CompGCN classifier TRN2 kernel v3 — bf16 + dma_gather batched gathers.

Launch A (node phase, per core):
  TAB = [nfW2 local ; Rrel ; nfW1 global] bf16 in DRAM.  Edges tgt-sorted
  into 128-node blocks (12 tiles); within each block lo/hi-sorted by src
  so each dma_gather hits one <=32768-row window (int16 indices).  Per
  4-block group: 3 dma_gathers (src-lo, src-hi, tgt+rel).  Batched DVE
  adds + gelu, one-hot per block in one DVE op, bf16 agg matmuls into
  PSUM.  GRU batched over all nodes at the end, then U12 table matmuls.

Launch B (edge classifier, per core):
  TU = [U1 global ; U2 global] bf16.  Edges bucketed by
  (src>=32768, tgt>=32768) so each 12-tile group needs 2 dma_gathers
  from fixed windows.  bf16 ef@W3 matmuls, batched adds/gelu, bf16
  transposes, batched out matmuls.  Host un-buckets the output.
"""
import sys

sys.path.insert(0, '/opt/trn_rl_repo')
sys.path.insert(0, '/root/.axon_site')

import numpy as np
import ml_dtypes
import concourse.bass as bass
import concourse.bacc as bacc
import concourse.mybir as mybir
import concourse.tile as tile
import concourse.bass_utils as bu
from concourse import library_config

BF16NP = ml_dtypes.bfloat16

# zero-egress container: never upload profiling artifacts
bu.upload_artifacts = lambda tmpdir: 'local://' + tmpdir

dt = mybir.dt
F32 = dt.float32
BF16 = dt.bfloat16
I16 = dt.int16

D = 128
NREL = 64
NCLS = 16
HALF = 32768            # int16 index window

# ---- full-size problem config (hardcoded per contract) ----
class CFG:
    N = 50000
    E = 500000
    NC = 8
    NPC = 6250              # nodes per core
    NB = 49                 # 128-node blocks per core
    NODES_PAD = 49 * 128    # 6272
    TPB = 12                # edge tiles per block (cap 1536 edges)
    LOT = 8                 # src-lo tiles per block
    NPAD_G = 50176          # global padded nodes (392*128)
    GB = 4                  # blocks per gather group (A)

    @classmethod
    def derived(cls):
        cls.HIT = cls.TPB - cls.LOT if cls.NPAD_G > HALF else 0
        if cls.NPAD_G <= HALF:
            cls.LOT = cls.TPB
        cls.BLK_CAP = cls.TPB * D
        cls.LOC = cls.LOT * D          # lo slots per block
        cls.HIC = cls.HIT * D
        cls.TABR_REL = cls.NODES_PAD
        cls.TABR_W1 = cls.NODES_PAD + NREL     # global rows base
        cls.TABROWS = cls.TABR_W1 + cls.NPAD_G
        cls.E2 = cls.E // cls.NC
        cls.TOT = cls.NC * cls.NB * cls.BLK_CAP


CFG.derived()

TRACE = False
LAST_EXEC_NS = {}
USE_BACC = True


def _make_nc(n_devices):
    cls = bacc.Bacc if USE_BACC else bass.Bass
    return cls("TRN2", target_bir_lowering=False, debug=False,
               num_devices=n_devices)


def _finalize(nc):
    if USE_BACC and not nc.is_finalized():
        nc.finalize()
    return nc


def _build_A(cfg):
    nc = _make_nc(cfg.NC)
    NB, TPB, NPAD_G, NODES_PAD = cfg.NB, cfg.TPB, cfg.NPAD_G, cfg.NODES_PAD
    LOT, HIT, LOC, HIC, GB = cfg.LOT, cfg.HIT, cfg.LOC, cfg.HIC, cfg.GB
    BC = cfg.BLK_CAP
    has_hi = HIT > 0

    # inputs (bf16 data, f32 biases, i16 indices)
    nfT = nc.dram_tensor("nfT", (D, NPAD_G), BF16, kind="ExternalInput")
    nfTl = nc.dram_tensor("nfTl", (D, NODES_PAD), BF16, kind="ExternalInput")
    W1mT = nc.dram_tensor("W1mT", (D, D), BF16, kind="ExternalInput")
    W2mT = nc.dram_tensor("W2mT", (D, D), BF16, kind="ExternalInput")
    RrelH = nc.dram_tensor("RrelH", (NREL, D), BF16, kind="ExternalInput")
    wihT = nc.dram_tensor("wihT", (D, 3 * D), BF16, kind="ExternalInput")
    whhT = nc.dram_tensor("whhT", (D, 3 * D), BF16, kind="ExternalInput")
    gb = nc.dram_tensor("gb", (D, 4), F32, kind="ExternalInput")
    W12cT = nc.dram_tensor("W12cT", (D, 2 * D), BF16, kind="ExternalInput")
    iotB = nc.dram_tensor("iotB", (D, D), BF16, kind="ExternalInput")
    ixsl = nc.dram_tensor("ixsl", (D, NB * LOC // 16), I16,
                          kind="ExternalInput")
    if has_hi:
        ixsh = nc.dram_tensor("ixsh", (D, NB * HIC // 16), I16,
                              kind="ExternalInput")
    ixtr = nc.dram_tensor("ixtr", (D, NB * 2 * BC // 16), I16,
                          kind="ExternalInput")
    trl = nc.dram_tensor("trl", (D, NB * TPB), BF16, kind="ExternalInput")
    # outputs
    U12s = nc.dram_tensor("U12s", (NODES_PAD, 2 * D), BF16,
                          kind="ExternalOutput")
    # scratch
    TAB = nc.dram_tensor("TAB", (cfg.TABROWS, D), BF16, kind="Internal")

    with tile.TileContext(nc) as tc:
        with tc.tile_pool(name="const", bufs=1) as cp:
            if not USE_BACC:
                nc.gpsimd.load_library(library_config.mlp)
            w1m = cp.tile([D, D], BF16, name="w1m")
            nc.sync.dma_start(w1m[:], W1mT[:])
            w2m = cp.tile([D, D], BF16, name="w2m")
            nc.sync.dma_start(w2m[:], W2mT[:])
            wih = cp.tile([D, 3 * D], BF16, name="wih")
            nc.sync.dma_start(wih[:], wihT[:])
            whh = cp.tile([D, 3 * D], BF16, name="whh")
            nc.sync.dma_start(whh[:], whhT[:])
            gbt = cp.tile([D, 4], F32, name="gbt")
            nc.sync.dma_start(gbt[:], gb[:])
            w12c = cp.tile([D, 2 * D], BF16, name="w12c")
            nc.sync.dma_start(w12c[:], W12cT[:])
            iot = cp.tile([D, D], BF16, name="iot")
            nc.sync.dma_start(iot[:], iotB[:])
            trlt = cp.tile([D, NB * TPB], BF16, name="trlt")
            nc.sync.dma_start(trlt[:], trl[:])
            # rel rows: route host Rrel through SBUF into TAB
            rrl = cp.tile([NREL, D], BF16, name="rrl")
            nc.sync.dma_start(rrl[:], RrelH[:])
            nc.sync.dma_start(TAB[cfg.TABR_REL:cfg.TABR_REL + NREL, :], rrl[:])
            # persistent accumulators
            aggAll = cp.tile([D, NODES_PAD], BF16, name="aggAll")
            nfuAll = cp.tile([D, NODES_PAD], BF16, name="nfuAll")
            nfl = cp.tile([D, NODES_PAD], BF16, name="nfl")
            nc.sync.dma_start(nfl[:], nfTl[:])

            # ---- Phase T: node tables (bf16) ----
            with tc.tile_pool(name="pt", bufs=3) as pt, \
                 tc.tile_pool(name="ps_t", bufs=2, space="PSUM") as ps_t:
                # global nfW1 rows, 4 chunks (512 nodes) per group
                for g4 in range(NPAD_G // 512):
                    ch = pt.tile([D, 512], BF16, name="ch")
                    nc.sync.dma_start(ch[:], nfT[:, g4 * 512:(g4 + 1) * 512])
                    pb = ps_t.tile([D, 512], F32, space="PSUM", name="pb")
                    for t in range(4):
                        nc.tensor.matmul(
                            out=pb[:, t * D:(t + 1) * D],
                            lhsT=ch[:, t * D:(t + 1) * D], rhs=w1m[:],
                            start=True, stop=True)
                    ob = pt.tile([D, 512], BF16, name="ob")
                    nc.scalar.copy(ob[:], pb[:])
                    base = cfg.TABR_W1 + g4 * 512
                    for t in range(4):
                        nc.sync.dma_start(
                            TAB[base + t * D:base + (t + 1) * D, :],
                            ob[:, t * D:(t + 1) * D])
                # local nfW2 rows, 4-chunk groups with remainder
                nloc = 0
                while nloc < NB:
                    k = min(4, NB - nloc)
                    ch = pt.tile([D, k * D], BF16, name="chl")
                    nc.sync.dma_start(
                        ch[:], nfTl[:, nloc * D:(nloc + k) * D])
                    pb = ps_t.tile([D, 512], F32, space="PSUM", name="pbl")
                    for t in range(k):
                        nc.tensor.matmul(
                            out=pb[:, t * D:(t + 1) * D],
                            lhsT=ch[:, t * D:(t + 1) * D], rhs=w2m[:],
                            start=True, stop=True)
                    ob = pt.tile([D, k * D], BF16, name="obl")
                    nc.scalar.copy(ob[:], pb[:, :k * D])
                    base = nloc * D
                    for t in range(k):
                        nc.sync.dma_start(
                            TAB[base + t * D:base + (t + 1) * D, :],
                            ob[:, t * D:(t + 1) * D])
                    nloc += k

            # ---- Phase E: per-block gathers (<=1024 idxs each) ----
            with tc.tile_pool(name="pg", bufs=2) as pg, \
                 tc.tile_pool(name="px", bufs=2) as px, \
                 tc.tile_pool(name="pe", bufs=2) as pe, \
                 tc.tile_pool(name="ps_agg", bufs=2, space="PSUM") as ps_agg:
                W_LO = min(HALF, NPAD_G)

                def gchunks(total):
                    out, off = [], 0
                    while off < total:
                        c = min(1024, total - off)
                        out.append((off, c))
                        off += c
                    return out

                for b in range(NB):
                    # src-lo (LOC idxs), src-hi (HIC), tgt (BC), rel (BC)
                    parts = []
                    glo = pg.tile([D, LOC], BF16, name="glo")
                    parts.append((glo, ixsl, b * LOC, LOC,
                                  cfg.TABR_W1, W_LO))
                    if has_hi:
                        ghi = pg.tile([D, HIC], BF16, name="ghi")
                        parts.append((ghi, ixsh, b * HIC, HIC,
                                      cfg.TABR_W1 + HALF, NPAD_G - HALF))
                    gtg = pg.tile([D, BC], BF16, name="gtg")
                    parts.append((gtg, ixtr, b * 2 * BC, BC, 0, cfg.TABR_W1))
                    grl = pg.tile([D, BC], BF16, name="grl")
                    parts.append((grl, ixtr, b * 2 * BC + BC, BC,
                                  0, cfg.TABR_W1))
                    for (otile, ixdram, ibase, total, tb0, tw) in parts:
                        for (off, cnt) in gchunks(total):
                            ixc = px.tile([128, 64], I16, name="ixc")
                            nc.sync.dma_start(
                                ixc[:, :cnt // 16],
                                ixdram[:, (ibase + off) // 16:
                                       (ibase + off + cnt) // 16])
                            nc.gpsimd.dma_gather(
                                otile[:, off:off + cnt].rearrange(
                                    "p (t f) -> p t f", t=cnt // D),
                                TAB[tb0:tb0 + tw, :], ixc[:, :cnt // 16],
                                cnt, cnt, D)
                    s_tr = pe.tile([D, BC], BF16, name="s_tr")
                    nc.vector.tensor_add(out=s_tr[:], in0=gtg[:], in1=grl[:])
                    smsg = pe.tile([D, BC], BF16, name="smsg")
                    nc.vector.tensor_add(
                        out=smsg[:, 0:LOC], in0=s_tr[:, 0:LOC], in1=glo[:])
                    if has_hi:
                        nc.vector.tensor_add(
                            out=smsg[:, LOC:BC], in0=s_tr[:, LOC:BC],
                            in1=ghi[:])
                    msg = pe.tile([D, BC], BF16, name="msg")
                    for q in range(0, TPB, 4):
                        w = min(4, TPB - q) * D
                        nc.scalar.activation(
                            msg[:, q * D:q * D + w],
                            smsg[:, q * D:q * D + w],
                            mybir.ActivationFunctionType.Gelu)
                    oh12 = pe.tile([D, BC], BF16, name="oh12")
                    nc.vector.tensor_tensor(
                        out=oh12[:].rearrange("p (t f) -> p t f", t=TPB),
                        in0=iot[:].unsqueeze(1).to_broadcast([D, TPB, D]),
                        in1=trlt[:, b * TPB:(b + 1) * TPB].unsqueeze(2)
                            .to_broadcast([D, TPB, D]),
                        op=mybir.AluOpType.is_equal)
                    agg = ps_agg.tile([D, D], F32, space="PSUM", name="agg")
                    for t in range(TPB):
                        nc.tensor.matmul(
                            out=agg[:], lhsT=msg[:, t * D:(t + 1) * D],
                            rhs=oh12[:, t * D:(t + 1) * D],
                            start=(t == 0), stop=(t == TPB - 1))
                    nc.vector.tensor_copy(
                        aggAll[:, b * D:(b + 1) * D], agg[:])

            # ---- Phase G: GRU batched over nodes ----
            with tc.tile_pool(name="pr", bufs=2) as pr, \
                 tc.tile_pool(name="ps_r", bufs=1, space="PSUM") as ps_r, \
                 tc.tile_pool(name="ps_z", bufs=1, space="PSUM") as ps_z, \
                 tc.tile_pool(name="ps_ni", bufs=1, space="PSUM") as ps_ni, \
                 tc.tile_pool(name="ps_nh", bufs=1, space="PSUM") as ps_nh:
                col = 0
                while col < NODES_PAD:
                    w = min(512, NODES_PAD - col)
                    sl = slice(col, col + w)
                    bank_r = ps_r.tile([D, 512], F32, space="PSUM",
                                       name="bank_r")
                    nc.tensor.matmul(out=bank_r[:, :w], lhsT=wih[:, 0:D],
                                     rhs=aggAll[:, sl], start=True, stop=False)
                    nc.tensor.matmul(out=bank_r[:, :w], lhsT=whh[:, 0:D],
                                     rhs=nfl[:, sl], start=False, stop=True)
                    bank_z = ps_z.tile([D, 512], F32, space="PSUM",
                                       name="bank_z")
                    nc.tensor.matmul(out=bank_z[:, :w], lhsT=wih[:, D:2 * D],
                                     rhs=aggAll[:, sl], start=True, stop=False)
                    nc.tensor.matmul(out=bank_z[:, :w], lhsT=whh[:, D:2 * D],
                                     rhs=nfl[:, sl], start=False, stop=True)
                    bank_ni = ps_ni.tile([D, 512], F32, space="PSUM",
                                         name="bank_ni")
                    nc.tensor.matmul(out=bank_ni[:, :w],
                                     lhsT=wih[:, 2 * D:3 * D],
                                     rhs=aggAll[:, sl], start=True, stop=True)
                    bank_nh = ps_nh.tile([D, 512], F32, space="PSUM",
                                         name="bank_nh")
                    nc.tensor.matmul(out=bank_nh[:, :w],
                                     lhsT=whh[:, 2 * D:3 * D],
                                     rhs=nfl[:, sl], start=True, stop=True)
                    r = pr.tile([D, 512], BF16, name="r")
                    nc.scalar.activation(
                        r[:, :w], bank_r[:, :w],
                        mybir.ActivationFunctionType.Sigmoid,
                        bias=gbt[:, 0:1])
                    z = pr.tile([D, 512], BF16, name="z")
                    nc.scalar.activation(
                        z[:, :w], bank_z[:, :w],
                        mybir.ActivationFunctionType.Sigmoid,
                        bias=gbt[:, 1:2])
                    t1 = pr.tile([D, 512], BF16, name="t1")
                    nc.vector.scalar_tensor_tensor(
                        out=t1[:, :w], in0=bank_nh[:, :w],
                        scalar=gbt[:, 3:4], in1=r[:, :w],
                        op0=mybir.AluOpType.add, op1=mybir.AluOpType.mult)
                    t2 = pr.tile([D, 512], F32, name="t2")
                    nc.vector.tensor_add(out=t2[:, :w], in0=t1[:, :w],
                                         in1=bank_ni[:, :w])
                    n_ = pr.tile([D, 512], BF16, name="n_")
                    nc.scalar.activation(
                        n_[:, :w], t2[:, :w],
                        mybir.ActivationFunctionType.Tanh,
                        bias=gbt[:, 2:3])
                    d1 = pr.tile([D, 512], BF16, name="d1")
                    nc.vector.tensor_sub(out=d1[:, :w], in0=nfl[:, sl],
                                         in1=n_[:, :w])
                    d2 = pr.tile([D, 512], BF16, name="d2")
                    nc.vector.tensor_mul(out=d2[:, :w], in0=z[:, :w],
                                         in1=d1[:, :w])
                    nc.vector.tensor_add(out=nfuAll[:, sl], in0=n_[:, :w],
                                         in1=d2[:, :w])
                    col += w

            # ---- Phase U: U12 tables ----
            with tc.tile_pool(name="pu", bufs=3) as pu, \
                 tc.tile_pool(name="ps_u", bufs=2, space="PSUM") as ps_u:
                for b in range(NB):
                    bank = ps_u.tile([D, 2 * D], F32, space="PSUM",
                                     name="bank")
                    nc.tensor.matmul(out=bank[:],
                                     lhsT=nfuAll[:, b * D:(b + 1) * D],
                                     rhs=w12c[:], start=True, stop=True)
                    ob = pu.tile([D, 2 * D], BF16, name="obu")
                    nc.vector.tensor_copy(ob[:], bank[:])
                    nc.sync.dma_start(U12s[b * D:(b + 1) * D, :], ob[:])

    return _finalize(nc)


def _build_B(cfg, tb):
    """tb: per-bucket tile counts [4] (static, data-dependent)."""
    nc = _make_nc(cfg.NC)
    NPAD_G = cfg.NPAD_G
    T2B = sum(tb)
    E2P = T2B * D
    TU = nc.dram_tensor("TU", (2 * NPAD_G, D), BF16, kind="ExternalInput")
    efT = nc.dram_tensor("efT", (D, E2P), BF16, kind="ExternalInput")
    W3cT = nc.dram_tensor("W3cT", (D, D), BF16, kind="ExternalInput")
    clsW2T = nc.dram_tensor("clsW2T", (D, NCLS), BF16, kind="ExternalInput")
    ixs = nc.dram_tensor("ixs", (D, E2P // 16), I16, kind="ExternalInput")
    ixt = nc.dram_tensor("ixt", (D, E2P // 16), I16, kind="ExternalInput")
    outB = nc.dram_tensor("outB", (NCLS, E2P), F32, kind="ExternalOutput")

    GT = 8   # tiles per gather group (<=1024 idxs)
    W_LO = min(HALF, NPAD_G)
    W_HI = NPAD_G - W_LO

    with tile.TileContext(nc) as tc:
        with tc.tile_pool(name="const", bufs=1) as cp:
            if not USE_BACC:
                nc.gpsimd.load_library(library_config.mlp)
            w3 = cp.tile([D, D], BF16, name="w3")
            nc.sync.dma_start(w3[:], W3cT[:])
            w2 = cp.tile([D, NCLS], BF16, name="w2")
            nc.sync.dma_start(w2[:], clsW2T[:])
            ident = cp.tile([D, D], BF16, name="ident")
            from concourse.masks import make_identity
            make_identity(nc, ident[:])

            with tc.tile_pool(name="pg", bufs=2) as pg, \
                 tc.tile_pool(name="pgx", bufs=2) as pgx, \
                 tc.tile_pool(name="pe", bufs=2) as pe, \
                 tc.tile_pool(name="po", bufs=3) as po, \
                 tc.tile_pool(name="ps_pre", bufs=2, space="PSUM") as ps_pre, \
                 tc.tile_pool(name="ps_ht", bufs=2, space="PSUM") as ps_ht, \
                 tc.tile_pool(name="ps_o", bufs=2, space="PSUM") as ps_o:
                toff = 0
                for q in range(4):
                    nt = tb[q]
                    if nt == 0:
                        continue
                    # bucket q = (src_hi<<1) | tgt_hi
                    sb = HALF if q >= 2 else 0
                    sw = W_HI if q >= 2 else W_LO
                    tbb = NPAD_G + (HALF if q % 2 else 0)
                    tw = W_HI if q % 2 else W_LO
                    t0 = 0
                    while t0 < nt:
                        k = min(GT, nt - t0)
                        kc = k * D
                        tg = toff + t0   # global tile offset
                        ixcs = pgx.tile([128, GT * 8], I16, name="ixcs")
                        nc.sync.dma_start(ixcs[:, :k * 8],
                                          ixs[:, tg * 8:(tg + k) * 8])
                        gs = pg.tile([D, GT * D], BF16, name="gs")
                        nc.gpsimd.dma_gather(
                            gs[:, :kc].rearrange("p (t f) -> p t f", t=k),
                            TU[sb:sb + sw, :], ixcs[:, :k * 8],
                            kc, kc, D)
                        ixct = pgx.tile([128, GT * 8], I16, name="ixct")
                        nc.sync.dma_start(ixct[:, :k * 8],
                                          ixt[:, tg * 8:(tg + k) * 8])
                        gt_ = pg.tile([D, GT * D], BF16, name="gt_")
                        nc.gpsimd.dma_gather(
                            gt_[:, :kc].rearrange("p (t f) -> p t f", t=k),
                            TU[tbb:tbb + tw, :], ixct[:, :k * 8],
                            kc, kc, D)
                        efb = pg.tile([D, GT * D], BF16, name="efb")
                        nc.sync.dma_start(efb[:, 0:kc],
                                          efT[:, tg * D:(tg + k) * D])
                        s12 = pe.tile([D, GT * D], BF16, name="s12")
                        nc.vector.tensor_add(out=s12[:, 0:kc],
                                             in0=gs[:, 0:kc],
                                             in1=gt_[:, 0:kc])
                        qq = 0
                        while qq < k:
                            kq = min(4, k - qq)
                            wq = kq * D
                            pre = ps_pre.tile([D, 512], F32, space="PSUM",
                                              name="pre")
                            for t in range(kq):
                                nc.tensor.matmul(
                                    out=pre[:, t * D:(t + 1) * D],
                                    lhsT=efb[:, (qq + t) * D:(qq + t + 1) * D],
                                    rhs=w3[:], start=True, stop=True)
                            hcin = pe.tile([D, 512], BF16, name="hcin")
                            nc.vector.tensor_add(
                                out=hcin[:, :wq],
                                in0=s12[:, qq * D:qq * D + wq],
                                in1=pre[:, :wq])
                            hc = pe.tile([D, 512], BF16, name="hc")
                            nc.scalar.activation(
                                hc[:, :wq], hcin[:, :wq],
                                mybir.ActivationFunctionType.Gelu)
                            hT = ps_ht.tile([D, 512], BF16, space="PSUM",
                                            name="hT")
                            for t in range(kq):
                                nc.tensor.transpose(
                                    out=hT[:, t * D:(t + 1) * D],
                                    in_=hc[:, t * D:(t + 1) * D],
                                    identity=ident[:])
                            hTs = pe.tile([D, 512], BF16, name="hTs")
                            nc.vector.tensor_copy(hTs[:, :wq], hT[:, :wq])
                            ob = ps_o.tile([NCLS, 512], F32, space="PSUM",
                                           name="obp")
                            nc.tensor.matmul(out=ob[:, :wq], lhsT=w2[:],
                                             rhs=hTs[:, :wq],
                                             start=True, stop=True)
                            os_ = po.tile([NCLS, 512], F32, name="os_")
                            nc.scalar.copy(os_[:, :wq], ob[:, :wq])
                            nc.sync.dma_start(
                                outB[:, (tg + qq) * D:(tg + qq) * D + wq],
                                os_[:, :wq])
                            qq += kq
                        t0 += k
                    toff += nt

    return _finalize(nc)


_CACHE = {}


def _get(name, builder, *args):
    if name not in _CACHE:
        _CACHE[name] = builder(*args)
    return _CACHE[name]


def _run(nc, in_maps, tag, n_cores):
    kw = {}
    if TRACE:
        import tempfile
        kw = dict(trace=True, tmpdir=tempfile.mkdtemp(prefix=f"gcn2_{tag}_"))
    res = bu.run_bass_kernel_spmd(nc, in_maps, core_ids=list(range(n_cores)),
                                  **kw)
    if TRACE:
        LAST_EXEC_NS[tag] = res.exec_time_ns
        LAST_EXEC_NS[tag + "_dir"] = kw["tmpdir"]
    return res.results


def _bf(x):
    return np.asarray(x, np.float32).astype(BF16NP)


def _wrap16(a):
    """flat idx stream -> [128, n/16] i16, 16-wrapped + replicated x8."""
    n = a.shape[0]
    w16 = np.ascontiguousarray(a.reshape(n // 16, 16).T.astype(np.int16))
    return np.tile(w16, (8, 1))


def kernel_impl(cfg, node_features, edge_features, edge_index,
                labels_for_rel_emb, rel_emb, msg_W, msg_b, gru_w_ih, gru_w_hh,
                gru_b_ih, gru_b_hh, cls_W1, cls_b1, cls_W2, cls_b2):
    NC, NPC, NB, TPB = cfg.NC, cfg.NPC, cfg.NB, cfg.TPB
    N, E, NPAD_G, NODES_PAD = cfg.N, cfg.E, cfg.NPAD_G, cfg.NODES_PAD
    BLK_CAP, E2 = cfg.BLK_CAP, cfg.E2
    LOT, LOC, HIC = cfg.LOT, cfg.LOC, cfg.HIC
    has_hi = cfg.HIT > 0

    nf = np.asarray(node_features, np.float32)
    ef = np.asarray(edge_features, np.float32)
    src = np.asarray(edge_index[0]).astype(np.int64)
    tgt = np.asarray(edge_index[1]).astype(np.int64)
    lab = np.asarray(labels_for_rel_emb).astype(np.int64)
    msg_W = np.asarray(msg_W, np.float32)
    cls_W1 = np.asarray(cls_W1, np.float32)

    # ---- host: sort edges by tgt block, then src-half within block ----
    tgt_blk_key = tgt  # sorting by tgt sorts by (core, block)
    order = np.argsort(tgt_blk_key, kind="stable")
    tgt_s = tgt[order]; src_s = src[order]; lab_s = lab[order]
    core = tgt_s // NPC
    blk = (tgt_s - core * NPC) // D
    key = core * NB + blk
    counts = np.bincount(key, minlength=NC * NB)
    assert counts.max() <= BLK_CAP, f"block overflow: {counts.max()}"
    if has_hi:
        is_hi = (src_s >= HALF).astype(np.int64)
    else:
        is_hi = np.zeros_like(src_s)
    order2 = np.lexsort((is_hi, key))
    key2 = key[order2]; src2 = src_s[order2]; tgt2 = tgt_s[order2]
    lab2 = lab_s[order2]; hi2 = is_hi[order2]; core2 = core[order2]
    blk2 = blk[order2]
    edge_orig = order[order2]          # original edge id per sorted pos
    lo_in_blk = np.bincount(key2[hi2 == 0], minlength=NC * NB)
    hi_in_blk = np.bincount(key2[hi2 == 1], minlength=NC * NB)
    assert lo_in_blk.max() <= LOC, f"lo overflow: {lo_in_blk.max()}"
    if has_hi:
        assert hi_in_blk.max() <= HIC, f"hi overflow: {hi_in_blk.max()}"
    gstart2 = np.searchsorted(key2 * 2 + hi2, np.arange(2 * NC * NB + 1))
    rank = np.arange(len(key2)) - gstart2[key2 * 2 + hi2]
    slot = np.where(hi2 == 0, rank, LOC + rank)
    gslot = key2 * BLK_CAP + slot
    TOT = cfg.TOT
    SRCI = np.zeros(TOT, np.int64)
    SRCI[gslot] = np.where(hi2 == 1, src2 - HALF, src2)
    TGTI = np.zeros(TOT, np.int64)
    TGTI[gslot] = tgt2 - core2 * NPC
    RELI = np.zeros(TOT, np.int64)
    RELI[gslot] = cfg.TABR_REL + lab2
    TRELP = np.full(TOT, -1.0, np.float32)
    TRELP[gslot] = (tgt2 - (core2 * NPC + blk2 * D)).astype(np.float32)

    nfT = np.zeros((D, NPAD_G), np.float32)
    nfT[:, :N] = nf.T
    W1mT = np.ascontiguousarray(msg_W[:, 0:D].T)
    W2mT = np.ascontiguousarray(msg_W[:, D:2 * D].T)
    Rrel = (np.asarray(rel_emb, np.float32) @ msg_W[:, 2 * D:3 * D].T
            + np.asarray(msg_b, np.float32)).astype(np.float32)
    wihT = np.ascontiguousarray(np.asarray(gru_w_ih, np.float32).T)
    whhT = np.ascontiguousarray(np.asarray(gru_w_hh, np.float32).T)
    bih = np.asarray(gru_b_ih, np.float32)
    bhh = np.asarray(gru_b_hh, np.float32)
    gbh = np.stack([bih[0:D] + bhh[0:D], bih[D:2 * D] + bhh[D:2 * D],
                    bih[2 * D:3 * D], bhh[2 * D:3 * D]], 1).astype(np.float32)
    W12cT = np.concatenate(
        [cls_W1[:, 0:D].T, cls_W1[:, D:2 * D].T], axis=1)
    iota = np.broadcast_to(np.arange(D, dtype=np.float32), (D, D)).copy()

    def lay_trl(a):
        return np.ascontiguousarray(
            a.reshape(NB, TPB, D).transpose(2, 0, 1).reshape(D, NB * TPB))

    in_maps_A = []
    for c in range(NC):
        lo = c * NB * BLK_CAP
        hi = lo + NB * BLK_CAP
        nfTl = np.zeros((D, NODES_PAD), np.float32)
        w = min(NODES_PAD, NPAD_G - c * NPC)
        nfTl[:, :w] = nfT[:, c * NPC:c * NPC + w]
        sc = SRCI[lo:hi].reshape(NB, BLK_CAP)
        srclo = np.ascontiguousarray(sc[:, 0:LOC]).reshape(-1)
        tr = np.concatenate(
            [TGTI[lo:hi].reshape(NB, BLK_CAP),
             RELI[lo:hi].reshape(NB, BLK_CAP)], axis=1).reshape(-1)
        im = {
            "nfT": _bf(nfT), "nfTl": _bf(nfTl),
            "W1mT": _bf(W1mT), "W2mT": _bf(W2mT), "RrelH": _bf(Rrel),
            "wihT": _bf(wihT), "whhT": _bf(whhT), "gb": gbh,
            "W12cT": _bf(W12cT), "iotB": _bf(iota),
            "ixsl": _wrap16(srclo), "ixtr": _wrap16(tr),
            "trl": _bf(lay_trl(TRELP[lo:hi])),
        }
        if has_hi:
            srchi = np.ascontiguousarray(sc[:, LOC:BLK_CAP]).reshape(-1)
            im["ixsh"] = _wrap16(srchi)
        in_maps_A.append(im)

    ncA = _get("A", _build_A, cfg)
    resA = _run(ncA, in_maps_A, "A", NC)

    # ---- host: assemble TU table ----
    b1 = np.asarray(cls_b1, np.float32)
    U1g = np.zeros((NPAD_G, D), np.float32)
    U2g = np.zeros((NPAD_G, D), np.float32)
    for c in range(NC):
        u = np.asarray(resA[c]["U12s"]).astype(np.float32)
        w = min(NPC, N - c * NPC)
        U1g[c * NPC:c * NPC + w] = u[:w, 0:D]
        U2g[c * NPC:c * NPC + w] = u[:w, D:2 * D]
    U1g += b1[None, :]
    TU = _bf(np.concatenate([U1g, U2g], axis=0))

    W3cT = np.ascontiguousarray(cls_W1[:, 2 * D:3 * D].T)
    clsW2T = np.ascontiguousarray(np.asarray(cls_W2, np.float32).T)

    # ---- B: bucket edges by (src_hi, tgt_hi) per core ----
    if has_hi:
        qid = ((src >= HALF).astype(np.int64) * 2
               + (tgt >= HALF).astype(np.int64)).reshape(NC, E2)
    else:
        qid = np.zeros((NC, E2), np.int64)
    bcounts = np.stack([(qid == q).sum(axis=1) for q in range(4)], axis=1)
    tb = [int(-(-bcounts[:, q].max() // D)) if bcounts[:, q].max() > 0 else 0
          for q in range(4)]
    T2B = sum(tb)
    E2P = T2B * D
    toffs = np.concatenate([[0], np.cumsum(tb)])

    in_maps_B = []
    cols_all = []
    for c in range(NC):
        sl = slice(c * E2, (c + 1) * E2)
        s2, t2 = src[sl], tgt[sl]
        cols = np.zeros(E2, np.int64)
        for q in range(4):
            m = qid[c] == q
            cols[m] = toffs[q] * D + np.arange(m.sum())
        cols_all.append(cols)
        efTc = np.zeros((D, E2P), np.float32)
        efTc[:, cols] = ef[sl].T
        si = np.zeros(E2P, np.int64)
        si[cols] = np.where(s2 >= HALF, s2 - HALF, s2) if has_hi else s2
        ti = np.zeros(E2P, np.int64)
        ti[cols] = np.where(t2 >= HALF, t2 - HALF, t2) if has_hi else t2
        in_maps_B.append({
            "TU": TU, "efT": _bf(efTc), "W3cT": _bf(W3cT),
            "clsW2T": _bf(clsW2T),
            "ixs": _wrap16(si), "ixt": _wrap16(ti),
        })

    ncB = _get("B", _build_B, cfg, tuple(tb))
    resB = _run(ncB, in_maps_B, "B", NC)

    b2 = np.asarray(cls_b2, np.float32)
    outs = []
    for c in range(NC):
        ob = np.asarray(resB[c]["outB"])
        outs.append(ob[:, cols_all[c]].T)
    out = np.concatenate(outs, axis=0)
    return np.ascontiguousarray((out + b2[None, :]).astype(np.float32))


def kernel(**inputs):
    return kernel_impl(CFG, **inputs)
